# revision 3
# baseline (speedup 1.0000x reference)
"""Multi-head attention (B=4, S=2048, D=1024, H=16) on 8 trn2 NeuronCores.

Sharding: batch x query-sequence-half. Core c handles batch c//2, query rows
[(c%2)*1024, (c%2+1)*1024), all 16 heads. K/V projections for the batch are
computed redundantly by the 2 cores sharing it (+25% flops, zero collectives).
Outputs are disjoint [1024, 1024] slices; the host concatenates.

v3 (per-core, feature-major "B" layout = [feature, seq]):
  prologue: Q^B = WqT.T @ xqT (+bq), all 8 dc chunks, borrowing pv PSUM slots.
  hp0 iters 0-7: K projection (32 N=512 groups, tb-major) + V projection
    (32 N=512 groups, tc-major) interleaved with scores(0, tc) -- all
    projections borrow the 4 "pv"-tag PSUM slots, which are otherwise idle
    until AV(0) starts.
  steady lap hp (iters 0..15): scores(hp, i); AV(hp-1, 8+i) for i<8;
    normalize(hp-1) at i==8; AV(hp, i-8) for i>=8.  AV is phase-shifted a
    half-lap behind scores, so exp tiles live ~8 iters (22-slot ring).
  AV with fused denominator: V stationary carries a 65th ones column
    (M=65), so den[s] = sum_t exp rides the same matmul stream into
    partition 64 of each head's own PSUM bank.  No dedicated den matmuls.
  normalize: den row -> DVE reciprocal -> row-broadcast via K=1 PE matmul
    into a borrowed (dead) "st" PSUM tile -> DVE multiply.  Even head
    writes o_all[0:64] directly; odd head lands in an SBUF staging tile
    and a SBUF->SBUF DMA shifts it to partitions 64:128 (DVE lanes cannot
    cross partitions; DMA can).
  epilogue: AV(7, 8..15), normalize(7) split by sb to overlap the
    out-projection (out = O^B.T @ W0T + b0 via K=1 ones-row matmul).

PSUM: st 4 banks (2 tiles x bufs 2, also borrowed for rbc broadcast and
out-projection accumulation) + pv 4 banks (bufs 4: Q/K/V projection
scratch during prologue/hp0, then per-(head,sb) AV accumulators) = 8.
"""

import numpy as np
import ml_dtypes

import concourse.bass as bass  # noqa: F401
import concourse.tile as tile
import concourse.mybir as mybir
from concourse import bacc
from concourse.bass_utils import run_bass_kernel_spmd

BF16 = mybir.dt.bfloat16
F32 = mybir.dt.float32
NP_BF16 = ml_dtypes.bfloat16

D = 1024          # d_model
S_CORE = 1024     # query rows per core
T = 2048          # key/value rows (full sequence)
H = 16            # heads
DK = 64           # head dim
KC = D // 128     # 8 contraction chunks
TC = T // 128     # 16 t-chunks
SB = S_CORE // 512  # 2 s-blocks of 512
DB = D // 512     # 2 feature blocks of 512
HP = H // 2       # 8 head pairs


def build(loop_n: int = 1):
    nc = bacc.Bacc("TRN2", target_bir_lowering=False, debug=False)

    xq = nc.dram_tensor("xq", [D, S_CORE], BF16, kind="ExternalInput")
    xk = nc.dram_tensor("xk", [D, T], BF16, kind="ExternalInput")
    xv = nc.dram_tensor("xv", [D, T], BF16, kind="ExternalInput")
    wq = nc.dram_tensor("wq", [D, D], BF16, kind="ExternalInput")
    wk = nc.dram_tensor("wk", [D, D], BF16, kind="ExternalInput")
    wv = nc.dram_tensor("wv", [D, D], BF16, kind="ExternalInput")
    w0 = nc.dram_tensor("w0", [D, D], BF16, kind="ExternalInput")
    bq = nc.dram_tensor("bq", [D], F32, kind="ExternalInput")
    bk = nc.dram_tensor("bk", [D], F32, kind="ExternalInput")
    b0e = nc.dram_tensor("b0e", [D], BF16, kind="ExternalInput")
    out = nc.dram_tensor("out", [S_CORE, D], F32, kind="ExternalOutput")

    with tile.TileContext(nc) as tc:
        def body():
            _body(nc, tc, xq, xk, xv, wq, wk, wv, w0, bq, bk, b0e, out)

        if loop_n == 1:
            body()
        else:
            hint = (
                mybir.EngineType.PE,
                mybir.EngineType.Activation,
                mybir.EngineType.DVE,
                mybir.EngineType.SP,
            )
            with tc.For_i(0, loop_n, 1, hint_engines=hint):
                body()

    nc.compile()
    return nc


def _body(nc, tc, xq, xk, xv, wq, wk, wv, w0, bq, bk, b0e, out):
    from contextlib import ExitStack

    with ExitStack() as ctx:
        persist = ctx.enter_context(tc.tile_pool(name="persist", bufs=1))
        q_all = persist.tile([128, KC, S_CORE], BF16, tag="q_all")
        k_all = persist.tile([128, KC, T], BF16, tag="k_all")
        v_all = persist.tile([128, TC, H, DK + 1], BF16, tag="v_all")
        o_all = persist.tile([128, KC, S_CORE], BF16, tag="o_all")
        # ones column per (tc, head): the fused softmax denominator
        nc.vector.memset(v_all[:, :, :, DK:DK + 1], 1.0)
        ones64 = persist.tile([128, 64], F32, tag="ones64")
        nc.vector.memset(ones64[:], 1.0)

        biasp = ctx.enter_context(tc.tile_pool(name="bias", bufs=1))
        bq_t = biasp.tile([128, KC], F32, tag="bq")
        nc.sync.dma_start(bq_t[:], bq.ap().rearrange("(c p) -> p c", p=128))
        bk_t = biasp.tile([128, KC], F32, tag="bk")
        nc.sync.dma_start(bk_t[:], bk.ap().rearrange("(c p) -> p c", p=128))

        b0_t = biasp.tile([1, D], BF16, tag="b0e")
        nc.sync.dma_start(b0_t[:], b0e.ap())
        onerow = biasp.tile([1, 128], BF16, tag="onerow")
        nc.vector.memset(onerow[:], 1.0)

        # weight slots: wk -> w0 share one slot; wv has its own
        wx2 = ctx.enter_context(tc.tile_pool(name="wx2", bufs=1))
        wk_t = wx2.tile([128, KC, D], BF16, tag="wkv", name="wk_t")
        nc.sync.dma_start(wk_t[:], wk.ap().rearrange("(c p) d -> p c d", p=128))
        wv_t = wx2.tile([128, KC, D], BF16, tag="wv", name="wv_t")
        nc.sync.dma_start(wv_t[:], wv.ap().rearrange("(c p) d -> p c d", p=128))

        # PSUM pools (created before prologue: Qproj borrows pv slots)
        psS = ctx.enter_context(tc.tile_pool(name="psS", bufs=2, space="PSUM"))
        pvp = ctx.enter_context(tc.tile_pool(name="pvp", bufs=4, space="PSUM"))

        # ---------------- prologue: Q projection (all chunks) ----------------
        with tc.tile_pool(name="prol", bufs=1) as prol:
            xq_t = prol.tile([128, KC, S_CORE], BF16, tag="xq")
            nc.sync.dma_start(xq_t[:], xq.ap().rearrange("(c p) s -> p c s", p=128))
            wq_t = prol.tile([128, KC, D], BF16, tag="wq")
            nc.sync.dma_start(wq_t[:], wq.ap().rearrange("(c p) d -> p c d", p=128))
            for dc in range(KC):
                for sb in range(SB):
                    ps = pvp.tile([128, 512], F32, tag="pv", name=f"qp{dc}_{sb}")
                    for kcc in range(KC):
                        nc.tensor.matmul(
                            ps[:],
                            wq_t[:, kcc, dc * 128:(dc + 1) * 128],
                            xq_t[:, kcc, sb * 512:(sb + 1) * 512],
                            start=(kcc == 0), stop=(kcc == KC - 1),
                        )
                    nc.vector.tensor_scalar_add(
                        q_all[:, dc, sb * 512:(sb + 1) * 512], ps[:],
                        bq_t[:, dc:dc + 1],
                    )

        # ---------------- main loop ----------------
        main_ctx = ExitStack()
        expp = main_ctx.enter_context(tc.tile_pool(name="expp", bufs=22))
        attn = main_ctx.enter_context(tc.tile_pool(name="attn", bufs=1))

        exps = {}       # (hp, tc, hh) -> expS tile [128, 1024]
        pvs = {}        # (hp, hh, sb) -> pv psum tile [128, 512]

        xk_r = xk.ap().rearrange("(c p) (q t) -> q p c t", p=128, t=512)
        xv_r = xv.ap().rearrange("(c p) (q t) -> q p c t", p=128, t=128)

        kp_state = {"g": 0, "xkq": None}

        def emit_kp_group():
            g = kp_state["g"]
            kp_state["g"] += 1
            tb, dc = divmod(g, KC)
            if dc == 0:
                xkq = wx2.tile([128, KC, 512], BF16, tag="xkq", bufs=2,
                               name=f"xk_q{tb}")
                nc.sync.dma_start(xkq[:], xk_r[tb])
                kp_state["xkq"] = xkq
            xkq = kp_state["xkq"]
            ps = pvp.tile([128, 512], F32, tag="pv", name=f"kp{g}")
            for kcc in range(KC):
                nc.tensor.matmul(
                    ps[:],
                    wk_t[:, kcc, dc * 128:(dc + 1) * 128],
                    xkq[:, kcc, :],
                    start=(kcc == 0), stop=(kcc == KC - 1),
                )
            nc.vector.tensor_scalar_add(
                k_all[:, dc, tb * 512:(tb + 1) * 512], ps[:],
                bk_t[:, dc:dc + 1],
            )

        def emit_vp_group(tcnk, g2):
            if g2 == 0:
                xvq = wx2.tile([128, KC, 128], BF16, tag="xvq", bufs=2,
                               name=f"xv_q{tcnk}")
                nc.sync.dma_start(xvq[:], xv_r[tcnk])
                emit_vp_group.cur = xvq
            xvq = emit_vp_group.cur
            ps = pvp.tile([128, 512], F32, tag="pv", name=f"vp{tcnk}_{g2}")
            for kcc in range(KC):
                nc.tensor.matmul(
                    ps[:],
                    xvq[:, kcc, :],
                    wv_t[:, kcc, g2 * 512:(g2 + 1) * 512],
                    start=(kcc == 0), stop=(kcc == KC - 1),
                )
            nc.vector.tensor_copy(
                v_all[:, tcnk, g2 * 8:(g2 + 1) * 8, 0:DK],
                ps[:].rearrange("p (h d) -> p h d", d=DK),
            )

        def emit_scores(hp, tcnk):
            dc = hp
            t_sl = slice(tcnk * 128, (tcnk + 1) * 128)
            sts = [
                psS.tile([128, 1024], F32, tag="st", name=f"st{hp}_{tcnk}_{hh}")
                for hh in range(2)
            ]
            for sb in range(SB):
                for hh in range(2):
                    p0 = hh * 64
                    nc.tensor.matmul(
                        sts[hh][:, sb * 512:(sb + 1) * 512],
                        k_all[p0:p0 + 64, dc, t_sl],
                        q_all[p0:p0 + 64, dc, sb * 512:(sb + 1) * 512],
                        start=True, stop=True,
                    )
            for hh in range(2):
                e = expp.tile([128, 1024], BF16, tag="expS",
                              name=f"e{hp}_{tcnk}_{hh}")
                nc.scalar.activation(
                    e[:], sts[hh][:],
                    mybir.ActivationFunctionType.Exp,
                    scale=0.125,
                )
                exps[(hp, tcnk, hh)] = e

        def emit_av(hp, tcnk):
            if tcnk == 0:
                for hh in range(2):
                    for sb in range(SB):
                        pvs[(hp, hh, sb)] = pvp.tile(
                            [128, 512], F32, tag="pv", name=f"pv{hp}_{hh}_{sb}")
            for sb in range(SB):
                s_sl = slice(sb * 512, (sb + 1) * 512)
                for hh in range(2):
                    nc.tensor.matmul(
                        pvs[(hp, hh, sb)][0:DK + 1, :],
                        v_all[:, tcnk, 2 * hp + hh, :],
                        exps[(hp, tcnk, hh)][:, s_sl],
                        start=(tcnk == 0), stop=(tcnk == TC - 1),
                        skip_group_check=True,
                    )
            del exps[(hp, tcnk, 0)]
            del exps[(hp, tcnk, 1)]

        def emit_normalize(hp, sbs=(0, 1)):
            dc = hp
            for sb in sbs:
                s_sl = slice(sb * 512, (sb + 1) * 512)
                rbc_ps = psS.tile([128, 1024], F32, tag="st",
                                  name=f"rbc{hp}_{sb}")
                for hh in range(2):
                    pv = pvs[(hp, hh, sb)]
                    dn = attn.tile([65, 512], F32, tag="den", bufs=1,
                                   name=f"dn{hp}_{sb}_{hh}")
                    nc.vector.tensor_copy(dn[DK:DK + 1, :], pv[DK:DK + 1, :])
                    rc = attn.tile([65, 512], F32, tag="recip", bufs=1,
                                   name=f"rc{hp}_{sb}_{hh}")
                    nc.vector.reciprocal(rc[DK:DK + 1, :], dn[DK:DK + 1, :])
                    nc.tensor.matmul(
                        rbc_ps[0:DK, hh * 512:(hh + 1) * 512],
                        ones64[DK:DK + 1, 0:DK],
                        rc[DK:DK + 1, :],
                        start=True, stop=True,
                        tile_position=(64, 0),
                        skip_group_check=True,
                    )
                for hh in range(2):
                    rb = attn.tile([64, 512], F32, tag="rbc", bufs=1,
                                   name=f"rb{hp}_{sb}_{hh}")
                    nc.vector.tensor_copy(
                        rb[:], rbc_ps[0:DK, hh * 512:(hh + 1) * 512])
                    pv = pvs.pop((hp, hh, sb))
                    if hh == 0:
                        nc.vector.tensor_mul(
                            o_all[0:DK, dc, s_sl], pv[0:DK, :], rb[:])
                    else:
                        stg = attn.tile([64, 512], BF16, tag="stg", bufs=2,
                                        name=f"stg{hp}_{sb}")
                        nc.vector.tensor_mul(stg[:], pv[0:DK, :], rb[:])
                        nc.sync.dma_start(o_all[DK:128, dc, s_sl], stg[:])

        # hp0 iters 0-7: K + V projections (borrow pv slots) + scores(0)
        for tcnk in range(8):
            for _ in range(2):
                emit_kp_group()
            emit_vp_group(2 * tcnk, 0)
            emit_vp_group(2 * tcnk, 1)
            emit_scores(0, tcnk)
            for _ in range(2):
                emit_kp_group()
            emit_vp_group(2 * tcnk + 1, 0)
            emit_vp_group(2 * tcnk + 1, 1)
        assert kp_state["g"] == 32

        # hp0 iters 8-15: scores(0) + AV(0, 0..7)
        for tcnk in range(8, TC):
            emit_scores(0, tcnk)
            emit_av(0, tcnk - 8)

        # w0 replaces wk in its slot; DMA overlaps the laps
        w0_t = wx2.tile([128, KC, D], BF16, tag="wkv", name="w0_t")
        nc.sync.dma_start(w0_t[:], w0.ap().rearrange("(c p) d -> p c d", p=128))

        # steady laps
        for hp in range(1, HP):
            for i in range(TC):
                if i == 8:
                    emit_normalize(hp - 1)
                emit_scores(hp, i)
                if i < 8:
                    emit_av(hp - 1, 8 + i)
                else:
                    emit_av(hp, i - 8)

        def emit_outproj(sc, db):
            # accumulate in a (dead) score-tag PSUM tile, region [:, 0:512]
            ps = psS.tile([128, 1024], F32, tag="st", name=f"oc{sc}_{db}")
            for dc in range(KC):
                nc.tensor.matmul(
                    ps[:, 0:512],
                    o_all[:, dc, sc * 128:(sc + 1) * 128],
                    w0_t[:, dc, db * 512:(db + 1) * 512],
                    start=(dc == 0), stop=False,
                )
            nc.tensor.matmul(
                ps[:, 0:512], onerow[:], b0_t[:, db * 512:(db + 1) * 512],
                start=False, stop=True,
            )
            ot = attn.tile([128, 512], F32, tag="ot", bufs=1, name=f"ot{sc}_{db}")
            nc.vector.tensor_copy(ot[:], ps[:, 0:512])
            nc.sync.dma_start(
                out.ap()[sc * 128:(sc + 1) * 128, db * 512:(db + 1) * 512],
                ot[:],
            )

        # tail: AV(7, 8..15); normalize(7) split by sb overlaps outproj
        for tcnk in range(8, TC):
            emit_av(7, tcnk)
        emit_normalize(7, sbs=(0,))
        for sc in range(4):
            for db in range(DB):
                emit_outproj(sc, db)
        emit_normalize(7, sbs=(1,))
        for sc in range(4, S_CORE // 128):
            for db in range(DB):
                emit_outproj(sc, db)

        main_ctx.close()


_NC_CACHE = {}


def _get_nc(loop_n=1):
    if loop_n not in _NC_CACHE:
        _NC_CACHE[loop_n] = build(loop_n)
    return _NC_CACHE[loop_n]


def _prep_in_maps(q, k, v, Wq, bq, Wk, bk, Wv, bv, W0, b0):
    def bt(x):  # bf16, C-contiguous transpose
        return np.ascontiguousarray(np.asarray(x, np.float32).T.astype(NP_BF16))

    wq_t, wk_t, wv_t, w0_t = bt(Wq), bt(Wk), bt(Wv), bt(W0)
    b0e = (
        np.asarray(b0, np.float64)
        + np.asarray(W0, np.float64) @ np.asarray(bv, np.float64)
    ).astype(np.float32).astype(NP_BF16)
    bq32 = np.ascontiguousarray(np.asarray(bq, np.float32))
    bk32 = np.ascontiguousarray(np.asarray(bk, np.float32))

    in_maps = []
    for c in range(8):
        b, hhalf = c // 2, c % 2
        sl = slice(hhalf * S_CORE, (hhalf + 1) * S_CORE)
        in_maps.append({
            "xq": bt(q[b, sl]),
            "xk": bt(k[b]),
            "xv": bt(v[b]),
            "wq": wq_t, "wk": wk_t, "wv": wv_t, "w0": w0_t,
            "bq": bq32, "bk": bk32, "b0e": b0e,
        })
    return in_maps


def kernel(q, k, v, mask, Wq, bq, Wk, bk, Wv, bv, W0, b0):
    nc = _get_nc(1)
    in_maps = _prep_in_maps(q, k, v, Wq, bq, Wk, bk, Wv, bv, W0, b0)
    res = run_bass_kernel_spmd(nc, in_maps, core_ids=list(range(8)))
    B, S = q.shape[0], q.shape[1]
    outv = np.empty((B, S, D), np.float32)
    for c in range(8):
        b, hhalf = c // 2, c % 2
        outv[b, hhalf * S_CORE:(hhalf + 1) * S_CORE, :] = res.results[c]["out"]
    return outv


# revision 12
# speedup vs baseline: 1.1218x; 1.1218x over previous
"""Multi-head attention (B=4, S=2048, D=1024, H=16) on 8 trn2 NeuronCores.

Sharding: batch x query-sequence-half. Core c handles batch c//2, query rows
[(c%2)*1024, (c%2+1)*1024), all 16 heads. K/V projections for the batch are
computed redundantly by the 2 cores sharing it (+25% flops, zero collectives).
Outputs are disjoint [1024, 1024] slices; the host concatenates.

v3 (per-core, feature-major "B" layout = [feature, seq]):
  prologue: Q^B = WqT.T @ xqT (+bq), all 8 dc chunks, borrowing pv PSUM slots.
  hp0 iters 0-7: K projection (32 N=512 groups, tb-major) + V projection
    (32 N=512 groups, tc-major) interleaved with scores(0, tc) -- all
    projections borrow the 4 "pv"-tag PSUM slots, which are otherwise idle
    until AV(0) starts.
  steady lap hp (iters 0..15): scores(hp, i); AV(hp-1, 8+i) for i<8;
    normalize(hp-1) at i==8; AV(hp, i-8) for i>=8.  AV is phase-shifted a
    half-lap behind scores, so exp tiles live ~8 iters (22-slot ring).
  AV with fused denominator: V stationary carries a 65th ones column
    (M=65), so den[s] = sum_t exp rides the same matmul stream into
    partition 64 of each head's own PSUM bank.  No dedicated den matmuls.
  normalize: den row -> DVE reciprocal -> row-broadcast via K=1 PE matmul
    into a borrowed (dead) "st" PSUM tile -> DVE multiply.  Even head
    writes o_all[0:64] directly; odd head lands in an SBUF staging tile
    and a SBUF->SBUF DMA shifts it to partitions 64:128 (DVE lanes cannot
    cross partitions; DMA can).
  epilogue: AV(7, 8..15), normalize(7) split by sb to overlap the
    out-projection (out = O^B.T @ W0T + b0 via K=1 ones-row matmul).

PSUM: st 4 banks (2 tiles x bufs 2, also borrowed for rbc broadcast and
out-projection accumulation) + pv 4 banks (bufs 4: Q/K/V projection
scratch during prologue/hp0, then per-(head,sb) AV accumulators) = 8.
"""

import numpy as np
import ml_dtypes

import concourse.bass as bass  # noqa: F401
import concourse.tile as tile
import concourse.mybir as mybir
from concourse import bacc
from concourse.bass_utils import run_bass_kernel_spmd

BF16 = mybir.dt.bfloat16
F32 = mybir.dt.float32
NP_BF16 = ml_dtypes.bfloat16

D = 1024          # d_model
S_CORE = 1024     # query rows per core
T = 2048          # key/value rows (full sequence)
H = 16            # heads
DK = 64           # head dim
KC = D // 128     # 8 contraction chunks
TC = T // 128     # 16 t-chunks
SB = S_CORE // 512  # 2 s-blocks of 512
DB = D // 512     # 2 feature blocks of 512
HP = H // 2       # 8 head pairs


def build(loop_n: int = 1):
    nc = bacc.Bacc("TRN2", target_bir_lowering=False, debug=False)

    xq = nc.dram_tensor("xq", [D, S_CORE], BF16, kind="ExternalInput")
    xk = nc.dram_tensor("xk", [D, T], BF16, kind="ExternalInput")
    xv = nc.dram_tensor("xv", [D, T], BF16, kind="ExternalInput")
    wq = nc.dram_tensor("wq", [D, D], BF16, kind="ExternalInput")
    wk = nc.dram_tensor("wk", [D, D], BF16, kind="ExternalInput")
    wv = nc.dram_tensor("wv", [D, D], BF16, kind="ExternalInput")
    w0 = nc.dram_tensor("w0", [D, D], BF16, kind="ExternalInput")
    bq = nc.dram_tensor("bq", [D], F32, kind="ExternalInput")
    bk = nc.dram_tensor("bk", [D], F32, kind="ExternalInput")
    b0e = nc.dram_tensor("b0e", [D], BF16, kind="ExternalInput")
    out = nc.dram_tensor("out", [S_CORE, D], F32, kind="ExternalOutput")

    with tile.TileContext(nc) as tc:
        def body():
            _body(nc, tc, xq, xk, xv, wq, wk, wv, w0, bq, bk, b0e, out)

        if loop_n == 1:
            body()
        else:
            hint = (
                mybir.EngineType.PE,
                mybir.EngineType.Activation,
                mybir.EngineType.DVE,
                mybir.EngineType.SP,
            )
            with tc.For_i(0, loop_n, 1, hint_engines=hint):
                body()

    nc.compile()
    return nc


def _body(nc, tc, xq, xk, xv, wq, wk, wv, w0, bq, bk, b0e, out):
    from contextlib import ExitStack

    with ExitStack() as ctx:
        persist = ctx.enter_context(tc.tile_pool(name="persist", bufs=1))
        q_all = persist.tile([128, KC, S_CORE], BF16, tag="q_all")
        k_all = persist.tile([128, KC, T], BF16, tag="k_all")
        v_all = persist.tile([128, TC, H, DK + 1], BF16, tag="v_all")
        o_all = persist.tile([128, KC, S_CORE], BF16, tag="o_all")
        # ones column per (tc, head): the fused softmax denominator
        nc.vector.memset(v_all[:, :, :, DK:DK + 1], 1.0)
        ones64 = persist.tile([128, 64], F32, tag="ones64")
        nc.vector.memset(ones64[:], 1.0)

        biasp = ctx.enter_context(tc.tile_pool(name="bias", bufs=1))
        bq_t = biasp.tile([128, KC], F32, tag="bq")
        nc.sync.dma_start(bq_t[:], bq.ap().rearrange("(c p) -> p c", p=128))
        bk_t = biasp.tile([128, KC], F32, tag="bk")
        nc.sync.dma_start(bk_t[:], bk.ap().rearrange("(c p) -> p c", p=128))

        b0_t = biasp.tile([1, D], BF16, tag="b0e")
        nc.sync.dma_start(b0_t[:], b0e.ap())
        onerow = biasp.tile([1, 128], BF16, tag="onerow")
        nc.vector.memset(onerow[:], 1.0)

        # weight slots: wk -> w0 share one slot; wv has its own.
        # Tiles are allocated here but their DMAs are emitted after xq/wq's
        # so the prologue Q projection isn't queued behind 4MB of weights.
        wx2 = ctx.enter_context(tc.tile_pool(name="wx2", bufs=1))
        wk_t = wx2.tile([128, KC, D], BF16, tag="wkv", name="wk_t")
        wv_t = wx2.tile([128, KC, D], BF16, tag="wv", name="wv_t")

        # PSUM pools (created before prologue: Qproj borrows pv slots)
        psS = ctx.enter_context(tc.tile_pool(name="psS", bufs=2, space="PSUM"))
        pvp = ctx.enter_context(tc.tile_pool(name="pvp", bufs=4, space="PSUM"))

        # ---------------- prologue: Q projection (all chunks) ----------------
        with tc.tile_pool(name="prol", bufs=1) as prol:
            xq_t = prol.tile([128, KC, S_CORE], BF16, tag="xq")
            # xq rides the Activation hwdge queue (idle in the prologue) so
            # it streams in parallel with wq on the sync queue
            nc.scalar.dma_start(xq_t[:], xq.ap().rearrange("(c p) s -> p c s", p=128))
            wq_t = prol.tile([128, KC, D], BF16, tag="wq")
            nc.sync.dma_start(wq_t[:], wq.ap().rearrange("(c p) d -> p c d", p=128))
            nc.sync.dma_start(wk_t[:], wk.ap().rearrange("(c p) d -> p c d", p=128))
            nc.sync.dma_start(wv_t[:], wv.ap().rearrange("(c p) d -> p c d", p=128))
            for dc in range(KC):
                for sb in range(SB):
                    ps = pvp.tile([128, 512], F32, tag="pv", name=f"qp{dc}_{sb}")
                    for kcc in range(KC):
                        nc.tensor.matmul(
                            ps[:],
                            wq_t[:, kcc, dc * 128:(dc + 1) * 128],
                            xq_t[:, kcc, sb * 512:(sb + 1) * 512],
                            start=(kcc == 0), stop=(kcc == KC - 1),
                        )
                    nc.vector.tensor_scalar_add(
                        q_all[:, dc, sb * 512:(sb + 1) * 512], ps[:],
                        bq_t[:, dc:dc + 1],
                    )

        # ---------------- main loop ----------------
        main_ctx = ExitStack()
        expp = main_ctx.enter_context(tc.tile_pool(name="expp", bufs=22))
        attn = main_ctx.enter_context(tc.tile_pool(name="attn", bufs=1))

        exps = {}       # (hp, tc, hh) -> expS tile [128, 1024]
        pvs = {}        # (hp, hh, sb) -> pv psum tile [128, 512]

        xk_r = xk.ap().rearrange("(c p) (q t) -> q p c t", p=128, t=512)
        xv_r = xv.ap().rearrange("(c p) (q t) -> q p c t", p=128, t=128)

        def proj_ps(name, borrow_st):
            # hp0 borrows the idle pv slots; laps borrow a dead st slot
            if borrow_st:
                t = psS.tile([128, 1024], F32, tag="st", name=name)
                return t[:, 0:512]
            return pvp.tile([128, 512], F32, tag="pv", name=name)[:]

        def emit_kp(tb, dcs, borrow_st=False):
            xkq = wx2.tile([128, KC, 512], BF16, tag="xkq", bufs=2,
                           name=f"xk_q{dcs[0]}_{tb}")
            nc.sync.dma_start(xkq[:], xk_r[tb])
            for dc in dcs:
                ps = proj_ps(f"kp{dc}_{tb}", borrow_st)
                for kcc in range(KC):
                    nc.tensor.matmul(
                        ps,
                        wk_t[:, kcc, dc * 128:(dc + 1) * 128],
                        xkq[:, kcc, :],
                        start=(kcc == 0), stop=(kcc == KC - 1),
                    )
                nc.vector.tensor_scalar_add(
                    k_all[:, dc, tb * 512:(tb + 1) * 512], ps,
                    bk_t[:, dc:dc + 1],
                )

        def emit_vp_group(tcnk, g2, borrow_st=False):
            xvq = wx2.tile([128, KC, 128], BF16, tag="xvq", bufs=2,
                           name=f"xv_q{tcnk}_{g2}")
            nc.sync.dma_start(xvq[:], xv_r[tcnk])
            ps = proj_ps(f"vp{tcnk}_{g2}", borrow_st)
            for kcc in range(KC):
                nc.tensor.matmul(
                    ps,
                    xvq[:, kcc, :],
                    wv_t[:, kcc, g2 * 512:(g2 + 1) * 512],
                    start=(kcc == 0), stop=(kcc == KC - 1),
                )
            nc.vector.tensor_copy(
                v_all[:, tcnk, g2 * 8:(g2 + 1) * 8, 0:DK],
                ps.rearrange("p (h d) -> p h d", d=DK),
            )

        def emit_scores(hp, tcnk):
            dc = hp
            t_sl = slice(tcnk * 128, (tcnk + 1) * 128)
            sts = [
                psS.tile([128, 1024], F32, tag="st", name=f"st{hp}_{tcnk}_{hh}")
                for hh in range(2)
            ]
            for sb in range(SB):
                for hh in range(2):
                    p0 = hh * 64
                    nc.tensor.matmul(
                        sts[hh][:, sb * 512:(sb + 1) * 512],
                        k_all[p0:p0 + 64, dc, t_sl],
                        q_all[p0:p0 + 64, dc, sb * 512:(sb + 1) * 512],
                        start=True, stop=True,
                    )
            for hh in range(2):
                e = expp.tile([128, 1024], BF16, tag="expS",
                              name=f"e{hp}_{tcnk}_{hh}")
                nc.scalar.activation(
                    e[:], sts[hh][:],
                    mybir.ActivationFunctionType.Exp,
                    scale=0.125,
                )
                exps[(hp, tcnk, hh)] = e

        def emit_av(hp, tcnk):
            if tcnk == 0:
                for hh in range(2):
                    for sb in range(SB):
                        pvs[(hp, hh, sb)] = pvp.tile(
                            [128, 512], F32, tag="pv", name=f"pv{hp}_{hh}_{sb}")
            for sb in range(SB):
                s_sl = slice(sb * 512, (sb + 1) * 512)
                for hh in range(2):
                    nc.tensor.matmul(
                        pvs[(hp, hh, sb)][0:DK + 1, :],
                        v_all[:, tcnk, 2 * hp + hh, :],
                        exps[(hp, tcnk, hh)][:, s_sl],
                        start=(tcnk == 0), stop=(tcnk == TC - 1),
                        skip_group_check=True,
                    )
            del exps[(hp, tcnk, 0)]
            del exps[(hp, tcnk, 1)]

        def emit_normalize(hp, sbs=(0, 1)):
            dc = hp
            for sb in sbs:
                s_sl = slice(sb * 512, (sb + 1) * 512)
                rbc_ps = psS.tile([128, 1024], F32, tag="st",
                                  name=f"rbc{hp}_{sb}")
                for hh in range(2):
                    pv = pvs[(hp, hh, sb)]
                    dn = attn.tile([65, 512], F32, tag="den", bufs=1,
                                   name=f"dn{hp}_{sb}_{hh}")
                    nc.vector.tensor_copy(dn[DK:DK + 1, :], pv[DK:DK + 1, :])
                    rc = attn.tile([65, 512], F32, tag="recip", bufs=1,
                                   name=f"rc{hp}_{sb}_{hh}")
                    nc.vector.reciprocal(rc[DK:DK + 1, :], dn[DK:DK + 1, :])
                    nc.tensor.matmul(
                        rbc_ps[0:DK, hh * 512:(hh + 1) * 512],
                        ones64[DK:DK + 1, 0:DK],
                        rc[DK:DK + 1, :],
                        start=True, stop=True,
                        tile_position=(64, 0),
                        skip_group_check=True,
                    )
                for hh in range(2):
                    rb = attn.tile([64, 512], F32, tag="rbc", bufs=1,
                                   name=f"rb{hp}_{sb}_{hh}")
                    nc.vector.tensor_copy(
                        rb[:], rbc_ps[0:DK, hh * 512:(hh + 1) * 512])
                    pv = pvs.pop((hp, hh, sb))
                    if hh == 0:
                        nc.vector.tensor_mul(
                            o_all[0:DK, dc, s_sl], pv[0:DK, :], rb[:])
                    else:
                        stg = attn.tile([64, 512], BF16, tag="stg", bufs=2,
                                        name=f"stg{hp}_{sb}")
                        nc.vector.tensor_mul(stg[:], pv[0:DK, :], rb[:])
                        nc.sync.dma_start(o_all[DK:128, dc, s_sl], stg[:])

        # hp0 iters 0-7: K proj dc0/dc1 + V proj heads 0-7 (borrow pv slots)
        # + scores(0).  Remaining projections ride the Act-paced laps.
        for tcnk in range(8):
            if tcnk % 2 == 0:
                emit_kp(tcnk // 2, (0, 1))
            emit_vp_group(2 * tcnk, 0)
            emit_scores(0, tcnk)
            emit_vp_group(2 * tcnk + 1, 0)

        # hp0 iters 8-15: AV(0, 0..7) + scores(0) + K proj chunk 2 in the
        # Act-paced slack (AV before scores so the PE never head-blocks on
        # the Act-dependent score matmuls)
        for tcnk in range(8, TC):
            emit_av(0, tcnk - 8)
            emit_scores(0, tcnk)
            if tcnk % 2 == 1:
                emit_kp((tcnk - 9) // 2, (2,), borrow_st=True)

        # steady laps; lap hp carries K proj chunk hp+2 (laps 1-5) and
        # V proj heads 8-15 for 4 t-chunks (laps 1-4), borrowing st slots
        for hp in range(1, HP):
            if hp == 6:
                # w0 replaces wk in its slot once the lap-5 K proj is done
                w0_t = wx2.tile([128, KC, D], BF16, tag="wkv", name="w0_t")
                nc.sync.dma_start(
                    w0_t[:], w0.ap().rearrange("(c p) d -> p c d", p=128))
            for i in range(TC):
                if i == 8:
                    emit_normalize(hp - 1)
                if i < 8:
                    emit_av(hp - 1, 8 + i)
                else:
                    emit_av(hp, i - 8)
                emit_scores(hp, i)
                if hp == 7 and i >= 8:
                    # lap 7 second half doubles up so AV(7) finishes
                    # with the lap and the tail is pure out-projection
                    emit_av(7, i)
                if hp <= 5 and i % 4 == 1:
                    emit_kp(i // 4, (hp + 2,), borrow_st=True)
                if hp <= 4 and i % 4 == 3:
                    emit_vp_group(4 * (hp - 1) + i // 4, 1, borrow_st=True)

        def emit_outproj(sc, db):
            # accumulate in a (dead) score-tag PSUM tile, region [:, 0:512]
            ps = psS.tile([128, 1024], F32, tag="st", name=f"oc{sc}_{db}")
            for dc in range(KC):
                nc.tensor.matmul(
                    ps[:, 0:512],
                    o_all[:, dc, sc * 128:(sc + 1) * 128],
                    w0_t[:, dc, db * 512:(db + 1) * 512],
                    start=(dc == 0), stop=False,
                )
            nc.tensor.matmul(
                ps[:, 0:512], onerow[:], b0_t[:, db * 512:(db + 1) * 512],
                start=False, stop=True,
            )
            ot = attn.tile([128, 512], F32, tag="ot", bufs=1, name=f"ot{sc}_{db}")
            nc.vector.tensor_copy(ot[:], ps[:, 0:512])
            nc.sync.dma_start(
                out.ap()[sc * 128:(sc + 1) * 128, db * 512:(db + 1) * 512],
                ot[:],
            )

        # tail: normalize(7) split by sb overlaps outproj
        emit_normalize(7, sbs=(0,))
        for sc in range(4):
            for db in range(DB):
                emit_outproj(sc, db)
        emit_normalize(7, sbs=(1,))
        for sc in range(4, S_CORE // 128):
            for db in range(DB):
                emit_outproj(sc, db)

        main_ctx.close()


_NC_CACHE = {}


def _get_nc(loop_n=1):
    if loop_n not in _NC_CACHE:
        _NC_CACHE[loop_n] = build(loop_n)
    return _NC_CACHE[loop_n]


def _prep_in_maps(q, k, v, Wq, bq, Wk, bk, Wv, bv, W0, b0):
    def bt(x):  # bf16, C-contiguous transpose
        return np.ascontiguousarray(np.asarray(x, np.float32).T.astype(NP_BF16))

    wq_t, wk_t, wv_t, w0_t = bt(Wq), bt(Wk), bt(Wv), bt(W0)
    b0e = (
        np.asarray(b0, np.float64)
        + np.asarray(W0, np.float64) @ np.asarray(bv, np.float64)
    ).astype(np.float32).astype(NP_BF16)
    bq32 = np.ascontiguousarray(np.asarray(bq, np.float32))
    bk32 = np.ascontiguousarray(np.asarray(bk, np.float32))

    in_maps = []
    for c in range(8):
        b, hhalf = c // 2, c % 2
        sl = slice(hhalf * S_CORE, (hhalf + 1) * S_CORE)
        in_maps.append({
            "xq": bt(q[b, sl]),
            "xk": bt(k[b]),
            "xv": bt(v[b]),
            "wq": wq_t, "wk": wk_t, "wv": wv_t, "w0": w0_t,
            "bq": bq32, "bk": bk32, "b0e": b0e,
        })
    return in_maps


def kernel(q, k, v, mask, Wq, bq, Wk, bk, Wv, bv, W0, b0):
    nc = _get_nc(1)
    in_maps = _prep_in_maps(q, k, v, Wq, bq, Wk, bk, Wv, bv, W0, b0)
    res = run_bass_kernel_spmd(nc, in_maps, core_ids=list(range(8)))
    B, S = q.shape[0], q.shape[1]
    outv = np.empty((B, S, D), np.float32)
    for c in range(8):
        b, hhalf = c // 2, c % 2
        outv[b, hhalf * S_CORE:(hhalf + 1) * S_CORE, :] = res.results[c]["out"]
    return outv


# revision 28
# speedup vs baseline: 1.1224x; 1.0005x over previous
"""Multi-head attention (B=4, S=2048, D=1024, H=16) on 8 trn2 NeuronCores.

Sharding: batch x query-sequence-half. Core c handles batch c//2, query rows
[(c%2)*1024, (c%2+1)*1024), all 16 heads. K/V projections for the batch are
computed redundantly by the 2 cores sharing it (+25% flops, zero collectives).
Outputs are disjoint [1024, 1024] slices; the host concatenates.

v3 (per-core, feature-major "B" layout = [feature, seq]):
  prologue: Q^B = WqT.T @ xqT (+bq), all 8 dc chunks, borrowing pv PSUM slots.
  hp0 iters 0-7: K projection (32 N=512 groups, tb-major) + V projection
    (32 N=512 groups, tc-major) interleaved with scores(0, tc) -- all
    projections borrow the 4 "pv"-tag PSUM slots, which are otherwise idle
    until AV(0) starts.
  steady lap hp (iters 0..15): scores(hp, i); AV(hp-1, 8+i) for i<8;
    normalize(hp-1) at i==8; AV(hp, i-8) for i>=8.  AV is phase-shifted a
    half-lap behind scores, so exp tiles live ~8 iters (22-slot ring).
  AV with fused denominator: V stationary carries a 65th ones column
    (M=65), so den[s] = sum_t exp rides the same matmul stream into
    partition 64 of each head's own PSUM bank.  No dedicated den matmuls.
  normalize: den row -> DVE reciprocal -> row-broadcast via K=1 PE matmul
    into a borrowed (dead) "st" PSUM tile -> DVE multiply.  Even head
    writes o_all[0:64] directly; odd head lands in an SBUF staging tile
    and a SBUF->SBUF DMA shifts it to partitions 64:128 (DVE lanes cannot
    cross partitions; DMA can).
  epilogue: AV(7, 8..15), normalize(7) split by sb to overlap the
    out-projection (out = O^B.T @ W0T + b0 via K=1 ones-row matmul).

PSUM: st 4 banks (2 tiles x bufs 2, also borrowed for rbc broadcast and
out-projection accumulation) + pv 4 banks (bufs 4: Q/K/V projection
scratch during prologue/hp0, then per-(head,sb) AV accumulators) = 8.
"""

import numpy as np
import ml_dtypes

import concourse.bass as bass  # noqa: F401
import concourse.tile as tile
import concourse.mybir as mybir
from concourse import bacc
from concourse.bass_utils import run_bass_kernel_spmd

BF16 = mybir.dt.bfloat16
F32 = mybir.dt.float32
NP_BF16 = ml_dtypes.bfloat16

D = 1024          # d_model
S_CORE = 1024     # query rows per core
T = 2048          # key/value rows (full sequence)
H = 16            # heads
DK = 64           # head dim
KC = D // 128     # 8 contraction chunks
TC = T // 128     # 16 t-chunks
SB = S_CORE // 512  # 2 s-blocks of 512
DB = D // 512     # 2 feature blocks of 512
HP = H // 2       # 8 head pairs


def build(loop_n: int = 1):
    nc = bacc.Bacc("TRN2", target_bir_lowering=False, debug=False)

    xq = nc.dram_tensor("xq", [D, S_CORE], BF16, kind="ExternalInput")
    xk = nc.dram_tensor("xk", [D, T], BF16, kind="ExternalInput")
    xv = nc.dram_tensor("xv", [D, T], BF16, kind="ExternalInput")
    wq = nc.dram_tensor("wq", [D, D], BF16, kind="ExternalInput")
    wk = nc.dram_tensor("wk", [D, D], BF16, kind="ExternalInput")
    wv = nc.dram_tensor("wv", [D, D], BF16, kind="ExternalInput")
    w0 = nc.dram_tensor("w0", [D, D], BF16, kind="ExternalInput")
    bq = nc.dram_tensor("bq", [D], F32, kind="ExternalInput")
    bk = nc.dram_tensor("bk", [D], F32, kind="ExternalInput")
    b0e = nc.dram_tensor("b0e", [D], BF16, kind="ExternalInput")
    out = nc.dram_tensor("out", [S_CORE, D], F32, kind="ExternalOutput")

    with tile.TileContext(nc) as tc:
        def body():
            _body(nc, tc, xq, xk, xv, wq, wk, wv, w0, bq, bk, b0e, out)

        if loop_n == 1:
            body()
        else:
            hint = (
                mybir.EngineType.PE,
                mybir.EngineType.Activation,
                mybir.EngineType.DVE,
                mybir.EngineType.SP,
            )
            with tc.For_i(0, loop_n, 1, hint_engines=hint):
                body()

    nc.compile()
    return nc


def _body(nc, tc, xq, xk, xv, wq, wk, wv, w0, bq, bk, b0e, out):
    from contextlib import ExitStack

    with ExitStack() as ctx:
        persist = ctx.enter_context(tc.tile_pool(name="persist", bufs=1))
        q_all = persist.tile([128, KC, S_CORE], BF16, tag="q_all")
        k_all = persist.tile([128, KC, T], BF16, tag="k_all")
        v_all = persist.tile([128, TC, H, DK + 1], BF16, tag="v_all")
        o_all = persist.tile([128, KC, S_CORE], BF16, tag="o_all")
        # ones column per (tc, head): the fused softmax denominator
        nc.vector.memset(v_all[:, :, :, DK:DK + 1], 1.0)
        ones64 = persist.tile([128, 64], F32, tag="ones64")
        nc.vector.memset(ones64[:], 1.0)

        biasp = ctx.enter_context(tc.tile_pool(name="bias", bufs=1))
        bq_t = biasp.tile([128, KC], F32, tag="bq")
        nc.sync.dma_start(bq_t[:], bq.ap().rearrange("(c p) -> p c", p=128))
        bk_t = biasp.tile([128, KC], F32, tag="bk")
        nc.sync.dma_start(bk_t[:], bk.ap().rearrange("(c p) -> p c", p=128))

        b0_t = biasp.tile([1, D], BF16, tag="b0e")
        nc.sync.dma_start(b0_t[:], b0e.ap())
        onerow = biasp.tile([1, 128], BF16, tag="onerow")
        nc.vector.memset(onerow[:], 1.0)

        # weight slots: wk -> w0 share one slot; wv has its own.
        # Tiles are allocated here but their DMAs are emitted after xq/wq's
        # so the prologue Q projection isn't queued behind 4MB of weights.
        wx2 = ctx.enter_context(tc.tile_pool(name="wx2", bufs=1))
        wk_t = wx2.tile([128, KC, D], BF16, tag="wkv", name="wk_t")
        wv_t = wx2.tile([128, KC, D], BF16, tag="wv", name="wv_t")

        # PSUM pools (created before prologue: Qproj borrows pv slots)
        psS = ctx.enter_context(tc.tile_pool(name="psS", bufs=2, space="PSUM"))
        pvp = ctx.enter_context(tc.tile_pool(name="pvp", bufs=4, space="PSUM"))

        # ---------------- prologue: Q projection (all chunks) ----------------
        with tc.tile_pool(name="prol", bufs=1) as prol:
            xq_t = prol.tile([128, KC, S_CORE], BF16, tag="xq")
            wq_t = prol.tile([128, KC, D], BF16, tag="wq")
            # halve the staging DMAs and loop sb-major so the first group
            # only waits for the sb0/dc0-3 halves (~6us, not ~11us)
            xq_rr = xq.ap().rearrange("(c p) s -> p c s", p=128)
            wq_rr = wq.ap().rearrange("(c p) d -> p c d", p=128)
            nc.scalar.dma_start(xq_t[:, :, 0:512], xq_rr[:, :, 0:512])
            nc.sync.dma_start(wq_t[:, :, 0:512], wq_rr[:, :, 0:512])
            nc.scalar.dma_start(xq_t[:, :, 512:1024], xq_rr[:, :, 512:1024])
            nc.sync.dma_start(wq_t[:, :, 512:1024], wq_rr[:, :, 512:1024])
            nc.scalar.dma_start(wk_t[:], wk.ap().rearrange("(c p) d -> p c d", p=128))
            nc.scalar.dma_start(wv_t[:], wv.ap().rearrange("(c p) d -> p c d", p=128))
            for sb in range(SB):
                for dc in range(KC):
                    ps = pvp.tile([128, 512], F32, tag="pv", name=f"qp{dc}_{sb}")
                    for kcc in range(KC):
                        nc.tensor.matmul(
                            ps[:],
                            wq_t[:, kcc, dc * 128:(dc + 1) * 128],
                            xq_t[:, kcc, sb * 512:(sb + 1) * 512],
                            start=(kcc == 0), stop=(kcc == KC - 1),
                        )
                    nc.vector.tensor_scalar_add(
                        q_all[:, dc, sb * 512:(sb + 1) * 512], ps[:],
                        bq_t[:, dc:dc + 1],
                    )

        # ---------------- main loop ----------------
        main_ctx = ExitStack()
        expp = main_ctx.enter_context(tc.tile_pool(name="expp", bufs=22))
        attn = main_ctx.enter_context(tc.tile_pool(name="attn", bufs=1))

        exps = {}       # (hp, tc, hh) -> expS tile [128, 1024]
        pvs = {}        # (hp, hh, sb) -> pv psum tile [128, 512]

        xk_r = xk.ap().rearrange("(c p) (q t) -> q p c t", p=128, t=512)
        xv_r = xv.ap().rearrange("(c p) (q t) -> q p c t", p=128, t=128)

        def proj_ps(name, borrow_st):
            # hp0 borrows the idle pv slots; laps borrow a dead st slot
            if borrow_st:
                t = psS.tile([128, 1024], F32, tag="st", name=name)
                return t[:, 0:512]
            return pvp.tile([128, 512], F32, tag="pv", name=name)[:]

        def emit_kp(tb, dcs, borrow_st=False):
            xkq = wx2.tile([128, KC, 512], BF16, tag="xkq", bufs=2,
                           name=f"xk_q{dcs[0]}_{tb}")
            nc.sync.dma_start(xkq[:], xk_r[tb])
            for dc in dcs:
                ps = proj_ps(f"kp{dc}_{tb}", borrow_st)
                for kcc in range(KC):
                    nc.tensor.matmul(
                        ps,
                        wk_t[:, kcc, dc * 128:(dc + 1) * 128],
                        xkq[:, kcc, :],
                        start=(kcc == 0), stop=(kcc == KC - 1),
                    )
                nc.vector.tensor_scalar_add(
                    k_all[:, dc, tb * 512:(tb + 1) * 512], ps,
                    bk_t[:, dc:dc + 1],
                )

        def emit_vp_group(tcnk, g2, borrow_st=False):
            xvq = wx2.tile([128, KC, 128], BF16, tag="xvq", bufs=2,
                           name=f"xv_q{tcnk}_{g2}")
            nc.sync.dma_start(xvq[:], xv_r[tcnk])
            ps = proj_ps(f"vp{tcnk}_{g2}", borrow_st)
            for kcc in range(KC):
                nc.tensor.matmul(
                    ps,
                    xvq[:, kcc, :],
                    wv_t[:, kcc, g2 * 512:(g2 + 1) * 512],
                    start=(kcc == 0), stop=(kcc == KC - 1),
                )
            nc.vector.tensor_copy(
                v_all[:, tcnk, g2 * 8:(g2 + 1) * 8, 0:DK],
                ps.rearrange("p (h d) -> p h d", d=DK),
            )

        def emit_scores(hp, tcnk):
            dc = hp
            t_sl = slice(tcnk * 128, (tcnk + 1) * 128)
            sts = [
                psS.tile([128, 1024], F32, tag="st", name=f"st{hp}_{tcnk}_{hh}")
                for hh in range(2)
            ]
            for sb in range(SB):
                for hh in range(2):
                    p0 = hh * 64
                    nc.tensor.matmul(
                        sts[hh][:, sb * 512:(sb + 1) * 512],
                        k_all[p0:p0 + 64, dc, t_sl],
                        q_all[p0:p0 + 64, dc, sb * 512:(sb + 1) * 512],
                        start=True, stop=True,
                    )
            for hh in range(2):
                e = expp.tile([128, 1024], BF16, tag="expS",
                              name=f"e{hp}_{tcnk}_{hh}")
                nc.scalar.activation(
                    e[:], sts[hh][:],
                    mybir.ActivationFunctionType.Exp,
                    scale=0.125,
                )
                exps[(hp, tcnk, hh)] = e

        def emit_av(hp, tcnk):
            if tcnk == 0:
                for hh in range(2):
                    for sb in range(SB):
                        pvs[(hp, hh, sb)] = pvp.tile(
                            [128, 512], F32, tag="pv", name=f"pv{hp}_{hh}_{sb}")
            for sb in range(SB):
                s_sl = slice(sb * 512, (sb + 1) * 512)
                for hh in range(2):
                    nc.tensor.matmul(
                        pvs[(hp, hh, sb)][0:DK + 1, :],
                        v_all[:, tcnk, 2 * hp + hh, :],
                        exps[(hp, tcnk, hh)][:, s_sl],
                        start=(tcnk == 0), stop=(tcnk == TC - 1),
                        skip_group_check=True,
                    )
            del exps[(hp, tcnk, 0)]
            del exps[(hp, tcnk, 1)]

        norm_st = {}    # (hp, sb) -> (rbc_ps st tile, [rc tiles])

        def emit_norm_prep(hp, sbs=(0, 1)):
            # den copy + reciprocal (DVE) emitted right after AV(hp, 15) so
            # the recip rows are ready before the broadcast matmuls issue
            for sb in sbs:
                rbc_ps = psS.tile([128, 1024], F32, tag="st",
                                  name=f"rbc{hp}_{sb}")
                rcs = []
                for hh in range(2):
                    pv = pvs[(hp, hh, sb)]
                    rc = attn.tile([65, 512], F32, tag="recip", bufs=2,
                                   name=f"rc{hp}_{sb}_{hh}")
                    nc.vector.reciprocal(rc[DK:DK + 1, :], pv[DK:DK + 1, :])
                    rcs.append(rc)
                norm_st[(hp, sb)] = (rbc_ps, rcs)

        def emit_normalize(hp, sbs=(0, 1)):
            dc = hp
            for sb in sbs:
                s_sl = slice(sb * 512, (sb + 1) * 512)
                rbc_ps, rcs = norm_st.pop((hp, sb))
                for hh in range(2):
                    nc.tensor.matmul(
                        rbc_ps[0:DK, hh * 512:(hh + 1) * 512],
                        ones64[DK:DK + 1, 0:DK],
                        rcs[hh][DK:DK + 1, :],
                        start=True, stop=True,
                        tile_position=(64, 0),
                        skip_group_check=True,
                    )
                for hh in range(2):
                    rb = attn.tile([64, 512], F32, tag="rbc", bufs=1,
                                   name=f"rb{hp}_{sb}_{hh}")
                    nc.vector.tensor_copy(
                        rb[:], rbc_ps[0:DK, hh * 512:(hh + 1) * 512])
                    pv = pvs.pop((hp, hh, sb))
                    if hh == 0:
                        nc.vector.tensor_mul(
                            o_all[0:DK, dc, s_sl], pv[0:DK, :], rb[:])
                    else:
                        stg = attn.tile([64, 512], BF16, tag="stg", bufs=2,
                                        name=f"stg{hp}_{sb}")
                        nc.vector.tensor_mul(stg[:], pv[0:DK, :], rb[:])
                        nc.sync.dma_start(o_all[DK:128, dc, s_sl], stg[:])

        # hp0 iters 0-7: K proj dc0/dc1 + V proj heads 0-7 (borrow pv slots)
        # + scores(0).  Remaining projections ride the Act-paced laps.
        for tcnk in range(8):
            if tcnk % 2 == 0:
                emit_kp(tcnk // 2, (0, 1))
            emit_vp_group(2 * tcnk, 0)
            emit_vp_group(2 * tcnk + 1, 0)
            emit_scores(0, tcnk)

        # hp0 iters 8-15: AV(0, 0..7) + scores(0) + K proj chunk 2 in the
        # Act-paced slack (AV before scores so the PE never head-blocks on
        # the Act-dependent score matmuls)
        for tcnk in range(8, TC):
            emit_av(0, tcnk - 8)
            if tcnk % 2 == 1:
                emit_kp((tcnk - 9) // 2, (2,), borrow_st=True)
            emit_scores(0, tcnk)

        # steady laps; lap hp carries K proj chunk hp+2 (laps 1-5) and
        # V proj heads 8-15 for 4 t-chunks (laps 1-4), borrowing st slots
        for hp in range(1, HP):
            if hp == 6:
                # w0 replaces wk in its slot once the lap-5 K proj is done
                w0_t = wx2.tile([128, KC, D], BF16, tag="wkv", name="w0_t")
                nc.sync.dma_start(
                    w0_t[:], w0.ap().rearrange("(c p) d -> p c d", p=128))
            for i in range(TC):
                if i < 8:
                    emit_av(hp - 1, 8 + i)
                    if i == 7:
                        emit_norm_prep(hp - 1)
                else:
                    if i == 8:
                        # broadcast matmuls are stall-free (recips were
                        # prepped at i==7) and must precede AV(hp, 0) in
                        # the PE FIFO, which waits on the freed pv slots
                        emit_normalize(hp - 1)
                    emit_av(hp, i - 8)
                if hp <= 5 and i % 4 == 1:
                    emit_kp(i // 4, (hp + 2,), borrow_st=True)
                if hp <= 4 and i % 4 == 3:
                    emit_vp_group(4 * (hp - 1) + i // 4, 1, borrow_st=True)
                emit_scores(hp, i)
                if hp == 7 and i >= 9:
                    # lap 7 second half doubles up (one iter behind the
                    # exp stream so the PE never waits on a fresh exp)
                    emit_av(7, i - 1)

        def emit_outproj(sc, db):
            # accumulate in a (dead) score-tag PSUM tile, region [:, 0:512]
            ps = psS.tile([128, 1024], F32, tag="st", name=f"oc{sc}_{db}")
            for dc in range(KC):
                nc.tensor.matmul(
                    ps[:, 0:512],
                    o_all[:, dc, sc * 128:(sc + 1) * 128],
                    w0_t[:, dc, db * 512:(db + 1) * 512],
                    start=(dc == 0), stop=False,
                )
            nc.tensor.matmul(
                ps[:, 0:512], onerow[:], b0_t[:, db * 512:(db + 1) * 512],
                start=False, stop=True,
            )
            ot = attn.tile([128, 512], F32, tag="ot", bufs=2, name=f"ot{sc}_{db}")
            nc.vector.tensor_copy(ot[:], ps[:, 0:512])
            nc.sync.dma_start(
                out.ap()[sc * 128:(sc + 1) * 128, db * 512:(db + 1) * 512],
                ot[:],
            )

        # tail: last AV chunk, then normalize(7) split by sb overlaps outproj
        emit_av(7, 15)
        emit_norm_prep(7, sbs=(0,))
        emit_normalize(7, sbs=(0,))
        for sc in range(4):
            for db in range(DB):
                emit_outproj(sc, db)
        emit_norm_prep(7, sbs=(1,))
        emit_normalize(7, sbs=(1,))
        for sc in range(4, S_CORE // 128):
            for db in range(DB):
                emit_outproj(sc, db)

        main_ctx.close()


_NC_CACHE = {}


def _get_nc(loop_n=1):
    if loop_n not in _NC_CACHE:
        _NC_CACHE[loop_n] = build(loop_n)
    return _NC_CACHE[loop_n]


def _prep_in_maps(q, k, v, Wq, bq, Wk, bk, Wv, bv, W0, b0):
    def bt(x):  # bf16, C-contiguous transpose
        return np.ascontiguousarray(np.asarray(x, np.float32).T.astype(NP_BF16))

    wq_t, wk_t, wv_t, w0_t = bt(Wq), bt(Wk), bt(Wv), bt(W0)
    b0e = (
        np.asarray(b0, np.float64)
        + np.asarray(W0, np.float64) @ np.asarray(bv, np.float64)
    ).astype(np.float32).astype(NP_BF16)
    bq32 = np.ascontiguousarray(np.asarray(bq, np.float32))
    bk32 = np.ascontiguousarray(np.asarray(bk, np.float32))

    in_maps = []
    for c in range(8):
        b, hhalf = c // 2, c % 2
        sl = slice(hhalf * S_CORE, (hhalf + 1) * S_CORE)
        in_maps.append({
            "xq": bt(q[b, sl]),
            "xk": bt(k[b]),
            "xv": bt(v[b]),
            "wq": wq_t, "wk": wk_t, "wv": wv_t, "w0": w0_t,
            "bq": bq32, "bk": bk32, "b0e": b0e,
        })
    return in_maps


def kernel(q, k, v, mask, Wq, bq, Wk, bk, Wv, bv, W0, b0):
    nc = _get_nc(1)
    in_maps = _prep_in_maps(q, k, v, Wq, bq, Wk, bk, Wv, bv, W0, b0)
    res = run_bass_kernel_spmd(nc, in_maps, core_ids=list(range(8)))
    B, S = q.shape[0], q.shape[1]
    outv = np.empty((B, S, D), np.float32)
    for c in range(8):
        b, hhalf = c // 2, c % 2
        outv[b, hhalf * S_CORE:(hhalf + 1) * S_CORE, :] = res.results[c]["out"]
    return outv


# revision 33
# speedup vs baseline: 1.2542x; 1.1174x over previous
"""Multi-head attention (B=4, S=2048, D=1024, H=16) on 8 trn2 NeuronCores.

Sharding: batch x query-sequence-half. Core c handles batch c//2, query rows
[(c%2)*1024, (c%2+1)*1024), all 16 heads. K/V projections for the batch are
computed redundantly by the 2 cores sharing it (+25% flops, zero collectives).
Outputs are disjoint [1024, 1024] slices; the host concatenates.

v11 (per-core, feature-major "B" layout = [feature, seq]):
  prologue: Q^B = WqT.T @ xqT (+bq), all 8 dc chunks, borrowing pv PSUM
    slots; xq/wq stream in halves on both hwdge queues so the first matmul
    issues at ~6us.
  hp0 iters 0-7: K projection dc0/dc1 + V projection heads 0-7 (N=512
    groups borrowing the 4 "pv"-tag PSUM slots, idle until AV(0) starts);
    iters 8-15 add K chunk dc2 in borrowed st slots.
  steady lap hp (iters 0..15): AV(hp-1, 8+i) for i<8 (ready exps first so
    the PE FIFO never head-blocks), then this lap's K/V projection groups
    (K chunk hp+2 on laps 1-5, V heads 8-15 on laps 1-4, st-slot borrows,
    xk/xv re-DMAed JIT), then scores(hp, i) last.  normalize(hp-1) recips
    are prepped at i==7 (DVE reads den straight from PSUM) so the i==8
    broadcast matmuls are stall-free and precede AV(hp, 0) in the FIFO.
    AV is phase-shifted a half-lap behind scores, so exp tiles live ~8
    iters (22-slot ring).  Lap 7's second half doubles AV one iter behind
    the exp stream so the tail is pure out-projection.
  AV with fused denominator: V stationary carries a 65th ones column
    (M=65), so den[s] = sum_t exp rides the same matmul stream into
    partition 64 of each head's own PSUM bank.  No dedicated den matmuls.
  normalize: den row -> DVE reciprocal -> row-broadcast via K=1 PE matmul
    into a borrowed (dead) "st" PSUM tile -> DVE multiply.  Even head
    writes o_all[0:64] directly; odd head lands in an SBUF staging tile
    and a SBUF->SBUF DMA shifts it to partitions 64:128 (DVE lanes cannot
    cross partitions; DMA can).
  epilogue: AV(7, 8..15), normalize(7) split by sb to overlap the
    out-projection (out = O^B.T @ W0T + b0 via K=1 ones-row matmul).

PSUM: st 4 banks (2 tiles x bufs 2, also borrowed for rbc broadcast and
out-projection accumulation) + pv 4 banks (bufs 4: Q/K/V projection
scratch during prologue/hp0, then per-(head,sb) AV accumulators) = 8.
"""

import numpy as np
import ml_dtypes

import concourse.bass as bass  # noqa: F401
import concourse.tile as tile
import concourse.mybir as mybir
from concourse import bacc
from concourse.bass_utils import run_bass_kernel_spmd

BF16 = mybir.dt.bfloat16
F32 = mybir.dt.float32
NP_BF16 = ml_dtypes.bfloat16

D = 1024          # d_model
S_CORE = 1024     # query rows per core
T = 2048          # key/value rows (full sequence)
H = 16            # heads
DK = 64           # head dim
KC = D // 128     # 8 contraction chunks
TC = T // 128     # 16 t-chunks
SB = S_CORE // 512  # 2 s-blocks of 512
DB = D // 512     # 2 feature blocks of 512
HP = H // 2       # 8 head pairs


def build(loop_n: int = 1):
    nc = bacc.Bacc("TRN2", target_bir_lowering=False, debug=False)

    xq = nc.dram_tensor("xq", [D, S_CORE], BF16, kind="ExternalInput")
    xk = nc.dram_tensor("xk", [D, T], BF16, kind="ExternalInput")
    xv = nc.dram_tensor("xv", [D, T], BF16, kind="ExternalInput")
    wq = nc.dram_tensor("wq", [D, D], BF16, kind="ExternalInput")
    wk = nc.dram_tensor("wk", [D, D], BF16, kind="ExternalInput")
    wv = nc.dram_tensor("wv", [D, D], BF16, kind="ExternalInput")
    w0 = nc.dram_tensor("w0", [D, D], BF16, kind="ExternalInput")
    bq = nc.dram_tensor("bq", [D], F32, kind="ExternalInput")
    bk = nc.dram_tensor("bk", [D], F32, kind="ExternalInput")
    b0e = nc.dram_tensor("b0e", [D], BF16, kind="ExternalInput")
    out = nc.dram_tensor("out", [S_CORE, D], F32, kind="ExternalOutput")

    with tile.TileContext(nc) as tc:
        def body():
            _body(nc, tc, xq, xk, xv, wq, wk, wv, w0, bq, bk, b0e, out)

        if loop_n == 1:
            body()
        else:
            hint = (
                mybir.EngineType.PE,
                mybir.EngineType.Activation,
                mybir.EngineType.DVE,
                mybir.EngineType.SP,
            )
            with tc.For_i(0, loop_n, 1, hint_engines=hint):
                body()

    nc.compile()
    return nc


def _body(nc, tc, xq, xk, xv, wq, wk, wv, w0, bq, bk, b0e, out):
    from contextlib import ExitStack

    with ExitStack() as ctx:
        persist = ctx.enter_context(tc.tile_pool(name="persist", bufs=1))
        q_all = persist.tile([128, KC, S_CORE], BF16, tag="q_all")
        # k chunks live in a 5-deep ring: chunk dc is produced ~2 laps
        # before lap dc reads it and is dead afterwards (frees 12KB SBUF)
        k_ring = {}
        v_all = persist.tile([128, TC, H, DK + 1], BF16, tag="v_all")
        o_all = persist.tile([128, KC, S_CORE], BF16, tag="o_all")
        # ones column per (tc, head): the fused softmax denominator
        nc.vector.memset(v_all[:, :, :, DK:DK + 1], 1.0)
        ones64 = persist.tile([128, 64], F32, tag="ones64")
        nc.vector.memset(ones64[:], 1.0)

        biasp = ctx.enter_context(tc.tile_pool(name="bias", bufs=1))
        bq_t = biasp.tile([128, KC], F32, tag="bq")
        nc.sync.dma_start(bq_t[:], bq.ap().rearrange("(c p) -> p c", p=128))
        bk_t = biasp.tile([128, KC], F32, tag="bk")
        nc.sync.dma_start(bk_t[:], bk.ap().rearrange("(c p) -> p c", p=128))

        b0_t = biasp.tile([1, D], BF16, tag="b0e")
        nc.sync.dma_start(b0_t[:], b0e.ap())
        onerow = biasp.tile([1, 128], BF16, tag="onerow")
        nc.vector.memset(onerow[:], 1.0)

        # weight slots: wk -> w0 share one slot; wv has its own.
        # Tiles are allocated here but their DMAs are emitted after xq/wq's
        # so the prologue Q projection isn't queued behind 4MB of weights.
        wx2 = ctx.enter_context(tc.tile_pool(name="wx2", bufs=1))
        wk_t = wx2.tile([128, KC, D], BF16, tag="wkv", name="wk_t")
        wv_t = wx2.tile([128, KC, D], BF16, tag="wv", name="wv_t")

        # PSUM pools (created before prologue: Qproj borrows pv slots)
        psS = ctx.enter_context(tc.tile_pool(name="psS", bufs=2, space="PSUM"))
        pvp = ctx.enter_context(tc.tile_pool(name="pvp", bufs=4, space="PSUM"))

        # ---------------- prologue: Q projection (all chunks) ----------------
        with tc.tile_pool(name="prol", bufs=1) as prol:
            xq_t = prol.tile([128, KC, S_CORE], BF16, tag="xq")
            wq_t = prol.tile([128, KC, D], BF16, tag="wq")
            # halve the staging DMAs and loop sb-major so the first group
            # only waits for the sb0/dc0-3 halves (~6us, not ~11us)
            xq_rr = xq.ap().rearrange("(c p) s -> p c s", p=128)
            wq_rr = wq.ap().rearrange("(c p) d -> p c d", p=128)
            nc.scalar.dma_start(xq_t[:, :, 0:512], xq_rr[:, :, 0:512])
            nc.sync.dma_start(wq_t[:, :, 0:512], wq_rr[:, :, 0:512])
            nc.scalar.dma_start(xq_t[:, :, 512:1024], xq_rr[:, :, 512:1024])
            nc.sync.dma_start(wq_t[:, :, 512:1024], wq_rr[:, :, 512:1024])
            nc.scalar.dma_start(wk_t[:], wk.ap().rearrange("(c p) d -> p c d", p=128))
            nc.scalar.dma_start(wv_t[:], wv.ap().rearrange("(c p) d -> p c d", p=128))
            for sb in range(SB):
                for dc in range(KC):
                    ps = pvp.tile([128, 512], F32, tag="pv", name=f"qp{dc}_{sb}")
                    for kcc in range(KC):
                        nc.tensor.matmul(
                            ps[:],
                            wq_t[:, kcc, dc * 128:(dc + 1) * 128],
                            xq_t[:, kcc, sb * 512:(sb + 1) * 512],
                            start=(kcc == 0), stop=(kcc == KC - 1),
                        )
                    nc.vector.tensor_scalar_add(
                        q_all[:, dc, sb * 512:(sb + 1) * 512], ps[:],
                        bq_t[:, dc:dc + 1],
                    )

        # ---------------- main loop ----------------
        main_ctx = ExitStack()
        expp = main_ctx.enter_context(tc.tile_pool(name="expp", bufs=23))
        attn = main_ctx.enter_context(tc.tile_pool(name="attn", bufs=1))

        exps = {}       # (hp, tc, hh) -> expS tile [128, 1024]
        pvs = {}        # (hp, hh, sb) -> pv psum tile [128, 512]

        xk_r = xk.ap().rearrange("(c p) (q t) -> q p c t", p=128, t=512)
        xv_r = xv.ap().rearrange("(c p) (q t) -> q p c t", p=128, t=128)

        def proj_ps(name, borrow_st):
            # hp0 borrows the idle pv slots; laps borrow a dead st slot
            if borrow_st:
                t = psS.tile([128, 1024], F32, tag="st", name=name)
                return t[:, 0:512]
            return pvp.tile([128, 512], F32, tag="pv", name=name)[:]

        def emit_kp(tb, dcs, borrow_st=False):
            xkq = wx2.tile([128, KC, 512], BF16, tag="xkq", bufs=3,
                           name=f"xk_q{dcs[0]}_{tb}")
            nc.sync.dma_start(xkq[:], xk_r[tb])
            for dc in dcs:
                if tb == 0:
                    k_ring[dc] = wx2.tile([128, T], BF16, tag="k_ring",
                                          bufs=5, name=f"k_c{dc}")
                ps = proj_ps(f"kp{dc}_{tb}", borrow_st)
                for kcc in range(KC):
                    nc.tensor.matmul(
                        ps,
                        wk_t[:, kcc, dc * 128:(dc + 1) * 128],
                        xkq[:, kcc, :],
                        start=(kcc == 0), stop=(kcc == KC - 1),
                    )
                nc.vector.tensor_scalar_add(
                    k_ring[dc][:, tb * 512:(tb + 1) * 512], ps,
                    bk_t[:, dc:dc + 1],
                )

        def emit_vp_group(tcnk, g2, borrow_st=False):
            xvq = wx2.tile([128, KC, 128], BF16, tag="xvq", bufs=3,
                           name=f"xv_q{tcnk}_{g2}")
            nc.sync.dma_start(xvq[:], xv_r[tcnk])
            ps = proj_ps(f"vp{tcnk}_{g2}", borrow_st)
            for kcc in range(KC):
                nc.tensor.matmul(
                    ps,
                    xvq[:, kcc, :],
                    wv_t[:, kcc, g2 * 512:(g2 + 1) * 512],
                    start=(kcc == 0), stop=(kcc == KC - 1),
                )
            nc.vector.tensor_copy(
                v_all[:, tcnk, g2 * 8:(g2 + 1) * 8, 0:DK],
                ps.rearrange("p (h d) -> p h d", d=DK),
            )

        def emit_scores(hp, tcnk):
            dc = hp
            t_sl = slice(tcnk * 128, (tcnk + 1) * 128)
            sts = [
                psS.tile([128, 1024], F32, tag="st", name=f"st{hp}_{tcnk}_{hh}")
                for hh in range(2)
            ]
            for sb in range(SB):
                for hh in range(2):
                    p0 = hh * 64
                    nc.tensor.matmul(
                        sts[hh][:, sb * 512:(sb + 1) * 512],
                        k_ring[dc][p0:p0 + 64, t_sl],
                        q_all[p0:p0 + 64, dc, sb * 512:(sb + 1) * 512],
                        start=True, stop=True,
                    )
            for hh in range(2):
                e = expp.tile([128, 1024], BF16, tag="expS",
                              name=f"e{hp}_{tcnk}_{hh}")
                nc.scalar.activation(
                    e[:], sts[hh][:],
                    mybir.ActivationFunctionType.Exp,
                    scale=0.125,
                )
                exps[(hp, tcnk, hh)] = e

        def emit_av(hp, tcnk):
            if tcnk == 0:
                for hh in range(2):
                    for sb in range(SB):
                        pvs[(hp, hh, sb)] = pvp.tile(
                            [128, 512], F32, tag="pv", name=f"pv{hp}_{hh}_{sb}")
            for sb in range(SB):
                s_sl = slice(sb * 512, (sb + 1) * 512)
                for hh in range(2):
                    nc.tensor.matmul(
                        pvs[(hp, hh, sb)][0:DK + 1, :],
                        v_all[:, tcnk, 2 * hp + hh, :],
                        exps[(hp, tcnk, hh)][:, s_sl],
                        start=(tcnk == 0), stop=(tcnk == TC - 1),
                        skip_group_check=True,
                    )
            del exps[(hp, tcnk, 0)]
            del exps[(hp, tcnk, 1)]

        norm_st = {}    # (hp, sb) -> (rbc_ps st tile, [rc tiles])

        def emit_norm_prep(hp, sbs=(0, 1)):
            # den copy + reciprocal (DVE) emitted right after AV(hp, 15) so
            # the recip rows are ready before the broadcast matmuls issue
            for sb in sbs:
                rbc_ps = psS.tile([128, 1024], F32, tag="st",
                                  name=f"rbc{hp}_{sb}")
                rcs = []
                for hh in range(2):
                    pv = pvs[(hp, hh, sb)]
                    rc = attn.tile([65, 512], F32, tag="recip", bufs=2,
                                   name=f"rc{hp}_{sb}_{hh}")
                    nc.vector.reciprocal(rc[DK:DK + 1, :], pv[DK:DK + 1, :])
                    rcs.append(rc)
                norm_st[(hp, sb)] = (rbc_ps, rcs)

        def emit_normalize(hp, sbs=(0, 1)):
            dc = hp
            for sb in sbs:
                s_sl = slice(sb * 512, (sb + 1) * 512)
                rbc_ps, rcs = norm_st.pop((hp, sb))
                for hh in range(2):
                    nc.tensor.matmul(
                        rbc_ps[0:DK, hh * 512:(hh + 1) * 512],
                        ones64[DK:DK + 1, 0:DK],
                        rcs[hh][DK:DK + 1, :],
                        start=True, stop=True,
                        tile_position=(64, 0),
                        skip_group_check=True,
                    )
                for hh in range(2):
                    rb = attn.tile([64, 512], F32, tag="rbc", bufs=1,
                                   name=f"rb{hp}_{sb}_{hh}")
                    nc.vector.tensor_copy(
                        rb[:], rbc_ps[0:DK, hh * 512:(hh + 1) * 512])
                    pv = pvs.pop((hp, hh, sb))
                    if hh == 0:
                        nc.vector.tensor_mul(
                            o_all[0:DK, dc, s_sl], pv[0:DK, :], rb[:])
                    else:
                        stg = attn.tile([64, 512], BF16, tag="stg", bufs=2,
                                        name=f"stg{hp}_{sb}")
                        nc.vector.tensor_mul(stg[:], pv[0:DK, :], rb[:])
                        nc.sync.dma_start(o_all[DK:128, dc, s_sl], stg[:])

        # hp0 iters 0-7: K proj dc0/dc1 + V proj heads 0-7 (borrow pv slots)
        # + scores(0).  Remaining projections ride the Act-paced laps.
        for tcnk in range(8):
            if tcnk % 2 == 0:
                emit_kp(tcnk // 2, (0, 1))
            emit_vp_group(2 * tcnk, 0)
            emit_vp_group(2 * tcnk + 1, 0)
            emit_scores(0, tcnk)

        # hp0 iters 8-15: AV(0, 0..7) + scores(0) + K proj chunk 2 in the
        # Act-paced slack (AV before scores so the PE never head-blocks on
        # the Act-dependent score matmuls)
        for tcnk in range(8, TC):
            emit_av(0, tcnk - 8)
            if tcnk % 2 == 1:
                emit_kp((tcnk - 9) // 2, (2,), borrow_st=True)
            emit_scores(0, tcnk)

        # steady laps; lap hp carries K proj chunk hp+2 (laps 1-5) and
        # V proj heads 8-15 for 4 t-chunks (laps 1-4), borrowing st slots
        for hp in range(1, HP):
            if hp == 6:
                # w0 replaces wk in its slot once the lap-5 K proj is done
                w0_t = wx2.tile([128, KC, D], BF16, tag="wkv", name="w0_t")
                nc.sync.dma_start(
                    w0_t[:], w0.ap().rearrange("(c p) d -> p c d", p=128))
            for i in range(TC):
                if i < 8:
                    emit_av(hp - 1, 8 + i)
                    if i == 7:
                        emit_norm_prep(hp - 1)
                else:
                    if i == 8:
                        # broadcast matmuls are stall-free (recips were
                        # prepped at i==7) and must precede AV(hp, 0) in
                        # the PE FIFO, which waits on the freed pv slots
                        emit_normalize(hp - 1)
                    emit_av(hp, i - 8)
                if hp <= 5 and i % 4 == 1:
                    emit_kp(i // 4, (hp + 2,), borrow_st=True)
                if hp <= 4 and i % 4 == 3:
                    emit_vp_group(4 * (hp - 1) + i // 4, 1, borrow_st=True)
                emit_scores(hp, i)
                if hp == 7 and i >= 9:
                    # lap 7 second half doubles up (one iter behind the
                    # exp stream so the PE never waits on a fresh exp)
                    emit_av(7, i - 1)

        def emit_outproj(sc, db):
            # accumulate in a (dead) score-tag PSUM tile, region [:, 0:512]
            ps = psS.tile([128, 1024], F32, tag="st", name=f"oc{sc}_{db}")
            for dc in range(KC):
                nc.tensor.matmul(
                    ps[:, 0:512],
                    o_all[:, dc, sc * 128:(sc + 1) * 128],
                    w0_t[:, dc, db * 512:(db + 1) * 512],
                    start=(dc == 0), stop=False,
                )
            nc.tensor.matmul(
                ps[:, 0:512], onerow[:], b0_t[:, db * 512:(db + 1) * 512],
                start=False, stop=True,
            )
            ot = attn.tile([128, 512], F32, tag="ot", bufs=2, name=f"ot{sc}_{db}")
            nc.vector.tensor_copy(ot[:], ps[:, 0:512])
            nc.sync.dma_start(
                out.ap()[sc * 128:(sc + 1) * 128, db * 512:(db + 1) * 512],
                ot[:],
            )

        # tail: last AV chunk, then normalize(7) split by sb overlaps outproj
        emit_av(7, 15)
        emit_norm_prep(7, sbs=(0,))
        emit_normalize(7, sbs=(0,))
        for sc in range(4):
            for db in range(DB):
                emit_outproj(sc, db)
        emit_norm_prep(7, sbs=(1,))
        emit_normalize(7, sbs=(1,))
        for sc in range(4, S_CORE // 128):
            for db in range(DB):
                emit_outproj(sc, db)

        main_ctx.close()


_NC_CACHE = {}


def _get_nc(loop_n=1):
    if loop_n not in _NC_CACHE:
        _NC_CACHE[loop_n] = build(loop_n)
    return _NC_CACHE[loop_n]


def _prep_in_maps(q, k, v, Wq, bq, Wk, bk, Wv, bv, W0, b0):
    def bt(x):  # bf16, C-contiguous transpose
        return np.ascontiguousarray(np.asarray(x, np.float32).T.astype(NP_BF16))

    wq_t, wk_t, wv_t, w0_t = bt(Wq), bt(Wk), bt(Wv), bt(W0)
    b0e = (
        np.asarray(b0, np.float64)
        + np.asarray(W0, np.float64) @ np.asarray(bv, np.float64)
    ).astype(np.float32).astype(NP_BF16)
    bq32 = np.ascontiguousarray(np.asarray(bq, np.float32))
    bk32 = np.ascontiguousarray(np.asarray(bk, np.float32))

    in_maps = []
    for c in range(8):
        b, hhalf = c // 2, c % 2
        sl = slice(hhalf * S_CORE, (hhalf + 1) * S_CORE)
        in_maps.append({
            "xq": bt(q[b, sl]),
            "xk": bt(k[b]),
            "xv": bt(v[b]),
            "wq": wq_t, "wk": wk_t, "wv": wv_t, "w0": w0_t,
            "bq": bq32, "bk": bk32, "b0e": b0e,
        })
    return in_maps


def kernel(q, k, v, mask, Wq, bq, Wk, bk, Wv, bv, W0, b0):
    nc = _get_nc(1)
    in_maps = _prep_in_maps(q, k, v, Wq, bq, Wk, bk, Wv, bv, W0, b0)
    res = run_bass_kernel_spmd(nc, in_maps, core_ids=list(range(8)))
    B, S = q.shape[0], q.shape[1]
    outv = np.empty((B, S, D), np.float32)
    for c in range(8):
        b, hhalf = c // 2, c % 2
        outv[b, hhalf * S_CORE:(hhalf + 1) * S_CORE, :] = res.results[c]["out"]
    return outv
